# revision 1
# baseline (speedup 1.0000x reference)
"""MLA (multi-head latent attention) distributed Bass kernel for TRN2.

Full inputs in / full output out. Sharding: 8 cores = 2 batches x 4 head-groups
(4 heads each). Per-core kernel computes the latent down-projections (duplicated
across the 4 cores of a batch), up-projects Q/K/V for its 4 heads, does causal
attention in a transposed [key, query] layout (scores^T from one matmul, exp on
ScalarE with the 1/sqrt(dk) folded into the activation scale, softmax
denominator via a ones-column appended to the V stationary), and a row-sharded
W_o partial product with W_o as the stationary operand (output in [m, q]
layout). Host sums the 4 partials per batch and transposes.

Per head, attention runs in two phases to decouple PE from ScalarE:
  phase 1: scores^T + exp for all (key-block, query-chunk) pairs -> expS tiles
  phase 2: AV sweep, accumulating [65, 512] psums (row 64 = denominator)
"""

import math
import numpy as np
import ml_dtypes

import concourse.bass as bass
import concourse.bacc as bacc
import concourse.mybir as mybir
import concourse.tile as tile
from concourse import bass_utils

BF16 = ml_dtypes.bfloat16

D_MODEL = 1024
N_HEADS = 16
D_K = 64
D_C = 256
B, S = 2, 2048

NH = 4          # heads per core
CH = 512        # query chunk (psum bank)
NCH = S // CH   # 4 query chunks
P = 128
NKB = S // P    # 16 key blocks
INV_SQRT_DK = 1.0 / math.sqrt(D_K)

_cached = None


def build_kernel():
    nc = bacc.Bacc("TRN2", debug=False, num_devices=8)
    dt = mybir.dt
    EXP = mybir.ActivationFunctionType.Exp
    DR = mybir.MatmulPerfMode.DoubleRow
    NKD = D_MODEL // P  # 8 d_model blocks

    xT_d = nc.dram_tensor("xT", [D_MODEL, S], dt.bfloat16, kind="ExternalInput")
    aq_d = nc.dram_tensor("aq", [P, NKD, NH * D_K], dt.bfloat16, kind="ExternalInput")
    wdkv_d = nc.dram_tensor("wdkv", [P, NKD, D_C], dt.bfloat16, kind="ExternalInput")
    wuk_d = nc.dram_tensor("wuk", [P, D_C // P, NH * D_K], dt.bfloat16, kind="ExternalInput")
    wuv_d = nc.dram_tensor("wuv", [P, D_C // P, NH * D_K], dt.bfloat16, kind="ExternalInput")
    wo_d = nc.dram_tensor("wo", [2, P, D_MODEL], dt.bfloat16, kind="ExternalInput")
    tri_d = nc.dram_tensor("tri", [P, P], dt.bfloat16, kind="ExternalInput")
    # output: y^T = (x @ ... @ W_o)^T in [m, q] layout
    yT_d = nc.dram_tensor("yT", [D_MODEL, S], dt.bfloat16, kind="ExternalOutput")

    with tile.TileContext(nc) as tc:
        with (
            tc.tile_pool(name="const", bufs=1) as const,
            tc.tile_pool(name="acts", bufs=1) as acts,
            tc.tile_pool(name="exps", bufs=1) as exps,
            tc.tile_pool(name="work", bufs=4) as work,
            tc.tile_pool(name="ps", bufs=4, space="PSUM") as ps,
            tc.tile_pool(name="psqk", bufs=2, space="PSUM") as psqk,
        ):
            xTv = xT_d.ap().rearrange("(n p) s -> n p s", p=P)
            # loads, first-needed first; xT split into column halves so the
            # first projection pass only waits on 2.5 MB
            wdkv = const.tile([P, NKD, D_C], dt.bfloat16, tag="wdkv")
            nc.sync.dma_start(wdkv[:], wdkv_d.ap())
            xT = [const.tile([P, S], dt.bfloat16, name=f"xT{n}", tag=f"xT{n}")
                  for n in range(NKD)]
            for n in range(NKD):
                nc.sync.dma_start(xT[n][:, 0:S // 2], xTv[n][:, 0:S // 2])
            wuk = const.tile([P, D_C // P, NH * D_K], dt.bfloat16, tag="wuk")
            nc.sync.dma_start(wuk[:], wuk_d.ap())
            aq = const.tile([P, NKD, NH * D_K], dt.bfloat16, tag="aq")
            nc.sync.dma_start(aq[:], aq_d.ap())
            tri = const.tile([P, P], dt.bfloat16, tag="tri")
            nc.sync.dma_start(tri[:], tri_d.ap())
            for n in range(NKD):
                nc.sync.dma_start(xT[n][:, S // 2:S], xTv[n][:, S // 2:S])
            wuv = const.tile([P, D_C // P, NH * D_K], dt.bfloat16, tag="wuv")
            nc.sync.dma_start(wuv[:], wuv_d.ap())
            wo = []
            for n in range(2):
                t = const.tile([P, D_MODEL], dt.bfloat16, name=f"wo{n}", tag=f"wo{n}")
                nc.sync.dma_start(t[:], wo_d.ap()[n])
                wo.append(t)

            # persistent activations
            ckvT = [acts.tile([P, S], dt.bfloat16, name=f"ckvT{i}", tag=f"ckvT{i}")
                    for i in range(2)]
            # fp8 DoubleRow packing: partition 32h+j = head h dims (2j, 2j+1)
            # in parity slots (host permutes aq/wuk columns accordingly)
            qTp = acts.tile([P, 2, S], dt.float8e4, name="qTp", tag="qTp")
            kTp = acts.tile([P, 2, S], dt.float8e4, name="kTp", tag="kTp")
            v_sb = [None] * NKB
            outT = [acts.tile([P, S], dt.bfloat16, name=f"outT{m}", tag=f"outT{m}")
                    for m in range(2)]

            # ---- single-psum projection pieces (interleavable) ----
            def emit_ckv(ch, half):
                sl = slice(ch * CH, (ch + 1) * CH)
                pp = ps.tile([P, CH], dt.float32, name="pp", tag="ps")
                for k in range(NKD):
                    nc.tensor.matmul(
                        pp[:], wdkv[:, k, half * P:(half + 1) * P],
                        xT[k][:, sl], start=(k == 0), stop=(k == NKD - 1))
                nc.vector.tensor_copy(ckvT[half][:, sl], pp[:])

            def emit_k(ch, m):
                sl = slice(ch * CH, (ch + 1) * CH)
                pp = ps.tile([P, CH], dt.float32, name="pp", tag="ps")
                for half in range(2):
                    nc.tensor.matmul(
                        pp[:], wuk[:, half, m * P:(m + 1) * P],
                        ckvT[half][:, sl], start=(half == 0), stop=(half == 1))
                nc.vector.tensor_copy(kTp[:, m, sl], pp[:])

            def emit_q(ch, m):
                sl = slice(ch * CH, (ch + 1) * CH)
                pp = ps.tile([P, CH], dt.float32, name="pp", tag="ps")
                for k in range(NKD):
                    nc.tensor.matmul(
                        pp[:], aq[:, k, m * P:(m + 1) * P],
                        xT[k][:, sl], start=(k == 0), stop=(k == NKD - 1))
                nc.vector.tensor_copy(qTp[:, m, sl], pp[:])

            def emit_v(kb):
                # V in [key, dim]: per head 64 dims + 64-wide ones block
                # (the ones columns replicate the softmax denominator to
                # psum rows 64:128 for free)
                vt = acts.tile([P, NH, 2 * D_K], dt.bfloat16,
                               name=f"v{kb}", tag=f"v{kb}")
                psv = ps.tile([P, NH * D_K], dt.float32, tag="ps")
                for half in range(2):
                    nc.tensor.matmul(
                        psv[:], ckvT[half][:, kb * P:(kb + 1) * P],
                        wuv[:, half, :], start=(half == 0), stop=(half == 1))
                nc.vector.tensor_copy(
                    vt[:, :, 0:D_K],
                    psv[:].rearrange("p (h d) -> p h d", h=NH))
                nc.gpsimd.memset(vt[:, :, D_K:2 * D_K], 1.0)
                v_sb[kb] = vt

            def emit_wo_mb(ch, mb):
                # yT[m, q] = sum_d wo[d, m] outT[d, q]: one (m, q-chunk) block
                sl = slice(ch * CH, (ch + 1) * CH)
                ysb = work.tile([P, CH], dt.bfloat16, tag="ysb")
                pp = ps.tile([P, CH], dt.float32, name="pp", tag="ps")
                for db in range(2):
                    nc.tensor.matmul(
                        pp[:], wo[db][:, mb * P:(mb + 1) * P],
                        outT[db][:, sl], start=(db == 0), stop=(db == 1))
                nc.vector.tensor_copy(ysb[:], pp[:])
                nc.sync.dma_start(yT_d.ap()[mb * P:(mb + 1) * P, sl], ysb[:])

            # first projection pass: chunks 0,1 (queries/keys 0:1024)
            for ch in (0, 1):
                for half in range(2):
                    emit_ckv(ch, half)
            for ch in (0, 1):
                for m in range(2):
                    emit_k(ch, m)
            for ch in (0, 1):
                for m in range(2):
                    emit_q(ch, m)

            # remaining pieces, interleaved into head 0's attention stream
            cp0_extras = {kb: [] for kb in range(10)}
            for kb in range(8):
                cp0_extras[kb].append(lambda kb=kb: emit_v(kb))
            for i, (ch, m) in enumerate(((2, 0), (2, 1), (3, 0), (3, 1))):
                cp0_extras[4 + i].append(lambda ch=ch, m=m: emit_q(ch, m))
            cp1_extras = {kb: [] for kb in range(18)}
            pieces = [lambda: emit_ckv(2, 0), lambda: emit_ckv(2, 1),
                      lambda: emit_k(2, 0), lambda: emit_k(2, 1),
                      lambda: emit_ckv(3, 0), lambda: emit_ckv(3, 1),
                      lambda: emit_k(3, 0), lambda: emit_k(3, 1)]
            for i, pc in enumerate(pieces):
                cp1_extras[i].append(pc)
            for kb in range(8, NKB):
                cp1_extras[kb].append(lambda kb=kb: emit_v(kb))

            # ---- attention: per head, chunk-pair major, QK/exp ahead of a
            # lagged AV sweep; denominator rows 64:128 of psav ----
            LAG = 2
            for h in range(NH):
                ht, off = divmod(h, 2)
                q_h = qTp[32 * h:32 * (h + 1), :, :]
                k_h = kTp[32 * h:32 * (h + 1), :, :]
                es_tiles = [None] * NKB
                psav = [None] * NCH

                def emit_qk(kb, cp):
                    q0 = P * kb       # first valid query for this key block
                    pq0 = 1024 * cp   # pair covers q in [pq0, pq0+1024)
                    if es_tiles[kb] is None:
                        es_tiles[kb] = exps.tile(
                            [P, S - q0], dt.bfloat16,
                            name=f"es{kb}", tag=f"es{kb}")
                    es = es_tiles[kb]
                    lo = max(q0, pq0)
                    pqk = psqk.tile([P, 2 * CH], dt.float32,
                                    name="pqk", tag="qk")
                    for ch in (2 * cp, 2 * cp + 1):
                        clo = max(q0, ch * CH)
                        if clo >= (ch + 1) * CH:
                            continue
                        nc.tensor.matmul(
                            pqk[:, clo - pq0:(ch + 1) * CH - pq0],
                            k_h[:, :, q0:q0 + P],
                            q_h[:, :, clo:(ch + 1) * CH],
                            start=True, stop=True, perf_mode=DR,
                            tile_position=(32 * h, 0))
                    nc.scalar.activation(
                        es[:, lo - q0:pq0 + 2 * CH - q0],
                        pqk[:, lo - pq0:2 * CH],
                        EXP, scale=INV_SQRT_DK)
                    if cp == kb // 8:
                        # mask the diagonal [128, 128] triangle (valid f >= p)
                        nc.vector.tensor_mul(es[:, 0:P], es[:, 0:P], tri[:])

                def emit_av(kb, cp):
                    q0 = P * kb
                    for c in (2 * cp, 2 * cp + 1):
                        if kb // 4 > c:
                            continue
                        n0 = max(q0 - CH * c, 0)
                        nc.tensor.matmul(
                            psav[c][:, n0:CH], v_sb[kb][:, h, :],
                            es_tiles[kb][:, CH * c + n0 - q0:
                                         CH * (c + 1) - q0],
                            start=(kb == 0), stop=(kb == 4 * c + 3))
                        if kb == 4 * c + 3:  # chunk done -> normalize
                            rb = work.tile([D_K, CH], dt.float32, tag="rb")
                            nc.vector.reciprocal(
                                rb[:], psav[c][D_K:2 * D_K, :])
                            nc.vector.tensor_mul(
                                outT[ht][off * D_K:(off + 1) * D_K,
                                         c * CH:(c + 1) * CH],
                                psav[c][0:D_K, :], rb[:])

                for cp in range(2):
                    for c in (2 * cp, 2 * cp + 1):
                        psav[c] = ps.tile([P, CH], dt.float32,
                                          name="psav", tag="ps")
                    kmax = 8 * cp + 8
                    if h == 0:
                        extras = cp0_extras if cp == 0 else cp1_extras
                    elif h == NH - 1 and cp == 1:
                        # W_o rides along head 3 cp1: chunks 0,1 are fully
                        # normalized after h3 cp0; chunk 2 after step 13
                        extras = {}
                        jobs = [(c, mb) for c in (0, 1)
                                for mb in range(D_MODEL // P)]
                        for kb, job in zip(range(1, 17), jobs):
                            extras.setdefault(kb, []).append(
                                lambda job=job: emit_wo_mb(job[0], job[1]))
                        for i, mb in enumerate(range(D_MODEL // P)):
                            extras.setdefault(14 + i % 4, []).append(
                                lambda mb=mb: emit_wo_mb(2, mb))
                    else:
                        extras = {}
                    for kb in range(kmax + LAG):
                        for fn in extras.get(kb, ()):
                            fn()
                        if kb < kmax:
                            emit_qk(kb, cp)
                        if kb >= LAG:
                            emit_av(kb - LAG, cp)
            for mb in range(D_MODEL // P):
                emit_wo_mb(3, mb)

    nc.compile()
    return nc


def _fold(w, p=P):
    # [K, M] -> [p, K/p, M] partition-major layout for contiguous DMA
    k, m = w.shape
    return np.ascontiguousarray(w.reshape(k // p, p, m).transpose(1, 0, 2))


# DoubleRow column permutation: M-col m<128 -> head m//32, dim 2*(m%32);
# m>=128 -> head (m-128)//32, dim 2*((m-128)%32)+1
_PERM = np.array([64 * ((m % 128) // 32) + 2 * (m % 32) + m // 128
                  for m in range(256)])


def _prep_inputs(x, W_dq, W_uq, W_dkv, W_uk, W_uv, W_o):
    tri = np.triu(np.ones((P, P), dtype=np.float32)).astype(BF16)  # f >= p
    in_maps = []
    for c in range(8):
        b, hg = divmod(c, 4)
        cs = slice(hg * NH * D_K, (hg + 1) * NH * D_K)
        aq = np.asarray(W_dq, np.float32) @ np.asarray(W_uq, np.float32)[:, cs]
        wuk = np.asarray(W_uk, np.float32)[:, cs]
        in_maps.append({
            "xT": np.ascontiguousarray(np.asarray(x)[b].T).astype(BF16),
            "aq": _fold(aq[:, _PERM].astype(BF16)),
            "wdkv": _fold(np.asarray(W_dkv).astype(BF16)),
            "wuk": _fold(wuk[:, _PERM].astype(BF16)),
            "wuv": _fold(np.asarray(W_uv)[:, cs].astype(BF16)),
            "wo": np.asarray(W_o)[cs, :].astype(BF16).reshape(2, P, D_MODEL),
            "tri": tri,
        })
    return in_maps


def run(inputs, trace=False, **kw):
    global _cached
    if _cached is None:
        _cached = build_kernel()
    in_maps = _prep_inputs(**inputs)
    res = bass_utils.run_bass_kernel_spmd(
        _cached, in_maps, core_ids=list(range(8)), trace=trace, **kw)
    ys = [res.results[c]["yT"].astype(np.float32) for c in range(8)]
    out = np.stack([
        (ys[0] + ys[1] + ys[2] + ys[3]).T,
        (ys[4] + ys[5] + ys[6] + ys[7]).T,
    ]).astype(np.float32)
    return out, res


def kernel(**inputs):
    out, _ = run(inputs)
    return out

